# revision 1
# baseline (speedup 1.0000x reference)
"""JointWiseFeedForward Trainium2 kernel.

Computes, for each of T=16 token positions t (feature-interleaved, f = o*16+t):
    y[:, :, o*16+t] = gelu(x_t @ W1_t.T) @ W2_t.T        (exact erf gelu)
with x [64,256,2048] fp32, W1 [512,128,16], W2 [128,512,16].

Strategy: data-parallel over the flattened 16384-token axis across 8 cores
(2048 tokens/core); weights replicated.  Per core, per (512-token group, t):
  PE transpose x chunks (stride-16 channel gather) -> X_t^T in SBUF,
  L1 matmul (W1 stationary, N=512, float32r full-rate fp32),
  exact-GELU on ScalarE evicting PSUM->SBUF,
  L2 accumulating matmul (W2 stationary) -> y_t^T in PSUM,
  PE transpose back to natural token-major layout,
  strided DVE evict into a [128, 2048] y tile, contiguous 1 MB DMAs out.
"""

import os
import sys

import numpy as np

try:
    import concourse.bass as bass
except ImportError:  # fresh grading dir: repo lives at a fixed path in the image
    sys.path.insert(0, "/opt/trn_rl_repo")
    import concourse.bass as bass

import concourse.mybir as mybir
import concourse.tile as tile
from concourse import bass_utils
from concourse.tile import add_dep_helper

NCORES = 8
B_TOTAL = 64 * 256          # 16384 flattened tokens
B_CORE = B_TOTAL // NCORES  # 2048
F = 2048
T = 16
CIN = 128                   # 2048 / 16
CFF = 512                   # 8192 / 16
COUT = 128
GROUP_TOK = 512             # tokens per inner group (one PSUM bank of fp32)
CHUNKS = GROUP_TOK // 128   # 4 x 128-token chunks per group
GROUPS = B_CORE // GROUP_TOK

F32R = mybir.dt.float32r    # full-rate fp32 matmul dtype on TRN2
F32 = mybir.dt.float32


def build_bass(n_iters: int = 1):
    nc = bass.Bass("TRN2")
    x = nc.dram_tensor("x", [B_CORE, F], F32R, kind="ExternalInput")
    w1t = nc.dram_tensor("w1t", [T, CIN, CFF], F32R, kind="ExternalInput")
    w2t = nc.dram_tensor("w2t", [T, 128, 4, COUT], F32R, kind="ExternalInput")
    ident = nc.dram_tensor("ident", [128, 128], F32R, kind="ExternalInput")
    y = nc.dram_tensor("y", [B_CORE, F], F32R, kind="ExternalOutput")

    with tile.TileContext(nc) as tc:
        with (
            tc.tile_pool(name="consts", bufs=1) as consts,
            tc.tile_pool(name="xg", bufs=5) as xpool,
            tc.tile_pool(name="yg", bufs=5) as ypool,
            tc.tile_pool(name="work", bufs=3) as work,
            tc.tile_pool(name="hbuf", bufs=2) as hpool,
            tc.tile_pool(name="ps_xt", bufs=1, space="PSUM") as ps_xt,
            tc.tile_pool(name="ps_h", bufs=2, space="PSUM") as ps_h,
            tc.tile_pool(name="ps_y", bufs=1, space="PSUM") as ps_y,
            tc.tile_pool(name="ps_yn", bufs=2, space="PSUM") as ps_yn,
        ):
            id_sb = consts.tile([128, 128], F32R, tag="ident")
            nc.sync.dma_start(out=id_sb, in_=ident[:, :])
            w1_sb = []
            w2_sb = []
            for t in range(T):
                w1tile = consts.tile([CIN, CFF], F32R, tag=f"w1_{t}")
                nc.gpsimd.dma_start(out=w1tile, in_=w1t[t])
                w2tile = consts.tile([128, 4, COUT], F32R, tag=f"w2_{t}")
                nc.gpsimd.dma_start(out=w2tile, in_=w2t[t])
                w1_sb.append(w1tile)
                w2_sb.append(w2tile)

            # Warm-up: touch every constant tile from the PE queue so each
            # weight-DMA wait lands on its own cheap transpose.  The HW has a
            # single sync-wait slot per instruction and walrus cannot split
            # >2 waits on the self-loading fp32 matmul path.
            warm = ps_xt.tile([128, GROUP_TOK], F32R, tag="pxt", name="warm")
            nc.tensor.transpose(warm[:, 0:128], id_sb, id_sb)
            for t in range(T):
                nc.tensor.transpose(warm[:, 0:128], w1_sb[t][:, 0:128], id_sb)
                nc.tensor.transpose(warm[:, 0:128], w2_sb[t][:, 0, :], id_sb)

            prev_gelu = None
            for g in list(range(GROUPS)) * n_iters:
                row0 = g * GROUP_TOK
                xts = []
                for bc in range(CHUNKS):
                    xt_ = xpool.tile([128, F], F32R, tag="xg")
                    nc.sync.dma_start(
                        out=xt_, in_=x[row0 + bc * 128 : row0 + (bc + 1) * 128, :]
                    )
                    xts.append(xt_)
                yts = [
                    ypool.tile([128, F], F32R, tag="yg", name=f"ytile_{g}_{i}")
                    for i in range(CHUNKS)
                ]

                for t in range(T):
                    # X_t^T: gather stride-16 channels of each 128-token chunk
                    p_xt = ps_xt.tile([128, GROUP_TOK], F32R, tag="pxt")
                    for bc in range(CHUNKS):
                        src = xts[bc].rearrange("p (c t) -> p c t", t=T)[:, :, t]
                        nc.tensor.transpose(
                            p_xt[:, bc * 128 : (bc + 1) * 128], src, id_sb
                        )
                    xt = work.tile([128, GROUP_TOK], F32R, tag="xt")
                    nc.vector.tensor_copy(out=xt, in_=p_xt)

                    # Absorb the p_h-release (previous gelu) wait on a PE nop
                    # so the first L1 matmul carries a single sync wait.
                    if prev_gelu is not None:
                        marker = nc.tensor.nop()
                        add_dep_helper(
                            marker.ins, prev_gelu.ins, True, "ph release prewait"
                        )

                    # L1: h^T chunks [128 ff, 512 tok] per output chunk oc.
                    # Two 2-bank PSUM halves so GELU(i) overlaps L1(i+1).
                    ht = hpool.tile([128, 4 * GROUP_TOK], F32R, tag="ht")
                    for half in range(2):
                        p_h = ps_h.tile([128, 2 * GROUP_TOK], F32, tag="ph")
                        for k in range(2):
                            oc = 2 * half + k
                            nc.tensor.matmul(
                                p_h[:, k * GROUP_TOK : (k + 1) * GROUP_TOK],
                                lhsT=w1_sb[t][:, oc * 128 : (oc + 1) * 128],
                                rhs=xt,
                                start=True,
                                stop=True,
                            )
                        prev_gelu = nc.scalar.activation(
                            out=ht[:, 2 * half * GROUP_TOK : 2 * (half + 1) * GROUP_TOK],
                            in_=p_h,
                            func=mybir.ActivationFunctionType.Gelu,
                        )

                    # L2: y_t^T [128 out, 512 tok], accumulate over ff chunks
                    p_y = ps_y.tile([COUT, GROUP_TOK], F32, tag="py")
                    for oc in range(4):
                        nc.tensor.matmul(
                            p_y,
                            lhsT=w2_sb[t][:, oc, :],
                            rhs=ht[:, oc * GROUP_TOK : (oc + 1) * GROUP_TOK],
                            start=(oc == 0),
                            stop=(oc == 3),
                        )
                    yt = work.tile([COUT, GROUP_TOK], F32R, tag="yt")
                    nc.vector.tensor_copy(out=yt, in_=p_y)

                    # back to token-major [128 tok, 128 out] and scatter into y tiles
                    p_yn = ps_yn.tile([128, GROUP_TOK], F32R, tag="pyn")
                    for bc in range(CHUNKS):
                        nc.tensor.transpose(
                            p_yn[:, bc * 128 : (bc + 1) * 128],
                            yt[:, bc * 128 : (bc + 1) * 128],
                            id_sb,
                        )
                    for bc in range(CHUNKS):
                        dst = yts[bc].rearrange("p (o t) -> p o t", t=T)[:, :, t]
                        nc.vector.tensor_copy(
                            out=dst, in_=p_yn[:, bc * 128 : (bc + 1) * 128]
                        )

                for bc in range(CHUNKS):
                    nc.scalar.dma_start(
                        out=y[row0 + bc * 128 : row0 + (bc + 1) * 128, :], in_=yts[bc]
                    )

    _split_matmul_waits(nc)
    return nc


def _split_matmul_waits(nc):
    """The fp32 self-loading matmul path has a single HW sync-wait slot and
    walrus cannot split multiple waits; hoist extras onto PE NoOps placed
    immediately before the matmul (same engine => program order preserved)."""
    for f in nc.m.functions:
        for bb in f.blocks:
            new = []
            changed = False
            for inst in bb.instructions:
                si = inst.sync_info
                if (
                    type(inst).__name__ != "InstNoOp"
                    and si is not None
                    and si.on_wait
                    and len(si.on_wait) > 1
                ):
                    waits = list(si.on_wait)
                    for w in waits[:-1]:
                        new.append(
                            mybir.InstNoOp(
                                name=nc.get_next_instruction_name(),
                                engine=inst.engine,
                                ins=[],
                                outs=[],
                                bass_nofuse=True,
                                sync_info=mybir.SyncInfo(on_wait=[w], on_update=[]),
                            )
                        )
                    inst.sync_info = mybir.SyncInfo(
                        on_wait=[waits[-1]], on_update=list(si.on_update)
                    )
                    changed = True
                new.append(inst)
            if changed:
                try:
                    bb.instructions[:] = new
                except TypeError:
                    bb.set_instructions(new)


def _prep_inputs(x, w1, w2):
    xf = np.ascontiguousarray(x.reshape(B_TOTAL, F).astype(np.float32, copy=False))
    # W1_t^T [c, o] = w1[o, c, t]
    w1t = np.ascontiguousarray(w1.transpose(2, 1, 0).astype(np.float32, copy=False))
    # w2 tile [f', oc, o] = w2[o, 128*oc + f', t]
    w2t = np.ascontiguousarray(
        w2.transpose(2, 1, 0)
        .reshape(T, 4, 128, COUT)
        .transpose(0, 2, 1, 3)
        .astype(np.float32, copy=False)
    )
    ident = np.eye(128, dtype=np.float32)
    return xf, w1t, w2t, ident


_RESULT_CACHE = {}


def kernel(**inputs):
    x = np.asarray(inputs["x"])
    w1 = np.asarray(inputs["w1"])
    w2 = np.asarray(inputs["w2"])
    xf, w1t, w2t, ident = _prep_inputs(x, w1, w2)

    nc = build_bass()
    in_maps = [
        {
            "x": xf[c * B_CORE : (c + 1) * B_CORE],
            "w1t": w1t,
            "w2t": w2t,
            "ident": ident,
        }
        for c in range(NCORES)
    ]
    res = bass_utils.run_bass_kernel_spmd(nc, in_maps, core_ids=list(range(NCORES)))
    if res.exec_time_ns is not None:
        print(f"HW exec time: {res.exec_time_ns} ns")
        _RESULT_CACHE["exec_time_ns"] = res.exec_time_ns
        _RESULT_CACHE["trace"] = res.instructions_and_trace
    y = np.concatenate([res.results[c]["y"] for c in range(NCORES)], axis=0)
    return y.reshape(64, 256, F)


if __name__ == "__main__":
    rng = np.random.default_rng(0)
    x = rng.standard_normal((64, 256, 2048), dtype=np.float32)
    w1 = (rng.standard_normal((512, 128, 16), dtype=np.float32) * 0.05).astype(
        np.float32
    )
    w2 = (rng.standard_normal((128, 512, 16), dtype=np.float32) * 0.05).astype(
        np.float32
    )
    y = kernel(x=x, w1=w1, w2=w2)
    print("ok", y.shape, float(np.abs(y).mean()))



# revision 2
# speedup vs baseline: 1.8450x; 1.8450x over previous
"""JointWiseFeedForward Trainium2 kernel.

Computes, for each of T=16 token positions t (feature-interleaved, f = o*16+t):
    y[:, :, o*16+t] = gelu(x_t @ W1_t.T) @ W2_t.T        (exact erf gelu)
with x [64,256,2048] fp32, W1 [512,128,16], W2 [128,512,16].

Strategy: data-parallel over the flattened 16384-token axis across 8 cores
(2048 tokens/core); weights replicated.  The host pre-transposes x into
channel-major slabs xT[t, cin, tok] (bf16) so the device runs a pure
matmul/activation pipeline with zero PE transposes and zero layout shuffles:

  per (t, 512-token group):
    L1: 4 bf16 matmuls (W1 chunks stationary) -> h^T in PSUM, fp32
    exact-GELU on ScalarE, PSUM -> SBUF bf16 (two 1024-wide activations)
    L2: 4 accumulating bf16 matmuls (W2 chunks stationary) -> y^T in PSUM
    DVE evicts y^T to SBUF bf16, DMA out channel-major; host re-interleaves.

Engine budget per core: PE 262k cycles @2.4GHz = 109us, ACT (gelu)
64*2*(1024+222) cyc @1.2GHz = 133us <- critical path, DVE 42us, DMA ~21MB.
L1(i+1) is emitted before L2(i) so the PE runs ahead while ScalarE chews
through gelu; PSUM = 3x[128,1024] h buffers + 2x[128,512] y = 8 banks.
"""

import sys

import numpy as np

try:
    import concourse.bass as bass
except ImportError:  # fresh grading dir: repo lives at a fixed path in the image
    sys.path.insert(0, "/opt/trn_rl_repo")
    import concourse.bass as bass

import ml_dtypes
import concourse.mybir as mybir
import concourse.tile as tile
from concourse import bass_utils

NCORES = 8
B_TOTAL = 64 * 256          # 16384 flattened tokens
B_CORE = B_TOTAL // NCORES  # 2048
F = 2048
T = 16
CIN = 128                   # 2048 / 16
CFF = 512                   # 8192 / 16
COUT = 128
GT = 512                    # tokens per inner group (one PSUM bank of fp32)
GROUPS = B_CORE // GT       # 4
ITERS = T * GROUPS          # 64

BF16 = mybir.dt.bfloat16
F32 = mybir.dt.float32
NPBF16 = ml_dtypes.bfloat16


def build_bass(n_iters: int = 1):
    nc = bass.Bass("TRN2")
    xT = nc.dram_tensor("xT", [T, CIN, B_CORE], BF16, kind="ExternalInput")
    w1t = nc.dram_tensor("w1t", [T, CIN, CFF], BF16, kind="ExternalInput")
    w2t = nc.dram_tensor("w2t", [T, 128, 4, COUT], BF16, kind="ExternalInput")
    yT = nc.dram_tensor("yT", [T, COUT, B_CORE], BF16, kind="ExternalOutput")

    with tile.TileContext(nc) as tc:
        with (
            tc.tile_pool(name="consts", bufs=1) as consts,
            tc.tile_pool(name="xg", bufs=2) as xpool,
            tc.tile_pool(name="htb", bufs=3) as htpool,
            tc.tile_pool(name="ysb", bufs=4) as ypool,
            tc.tile_pool(name="ps_h", bufs=3, space="PSUM") as ps_h,
            tc.tile_pool(name="ps_y", bufs=2, space="PSUM") as ps_y,
        ):
            w1_sb = {}
            w2_sb = {}
            xts = {}

            def load_weights(t):
                w1tile = consts.tile([CIN, CFF], BF16, tag=f"w1_{t}", name=f"w1s_{t}")
                nc.sync.dma_start(out=w1tile, in_=w1t[t])
                w2tile = consts.tile([128, 4, COUT], BF16, tag=f"w2_{t}", name=f"w2s_{t}")
                nc.sync.dma_start(out=w2tile, in_=w2t[t])
                w1_sb[t] = w1tile
                w2_sb[t] = w2tile

            def load_x(t):
                xt_ = xpool.tile([CIN, B_CORE], BF16, tag="xg", name=f"xt_{t}")
                nc.sync.dma_start(out=xt_, in_=xT[t])
                xts[t] = xt_

            # Preamble: first x slab + first two t's weights.
            load_x(0)
            load_weights(0)
            load_weights(1)

            sched = list(range(ITERS)) * n_iters

            def emit_l1(idx):
                """L1 matmuls for schedule position idx; returns PSUM halves."""
                i = sched[idx]
                t, g = i // GROUPS, i % GROUPS
                if g == 0 and idx < ITERS:  # prefetch only on the first pass
                    if t + 1 < T:
                        load_x(t + 1)
                    if t + 3 < T:
                        load_weights(t + 3)
                ph_a = ps_h.tile([128, 2 * GT], F32, tag="ph", name=f"ph_a_{idx}")
                ph_b = ps_h.tile([128, 2 * GT], F32, tag="ph", name=f"ph_b_{idx}")
                rhs = xts[t][:, g * GT : (g + 1) * GT]
                for k, ph in ((0, ph_a), (1, ph_a), (2, ph_b), (3, ph_b)):
                    nc.tensor.matmul(
                        ph[:, (k % 2) * GT : (k % 2 + 1) * GT],
                        lhsT=w1_sb[t][:, k * 128 : (k + 1) * 128],
                        rhs=rhs,
                        start=True,
                        stop=True,
                    )
                return ph_a, ph_b

            load_weights(2)
            ph_cur = emit_l1(0)
            for idx in range(len(sched)):
                i = sched[idx]
                t, g = i // GROUPS, i % GROUPS
                ph_a, ph_b = ph_cur

                ht = htpool.tile([128, 4 * GT], BF16, tag="ht", name=f"ht_{idx}")
                nc.scalar.activation(
                    out=ht[:, 0 : 2 * GT],
                    in_=ph_a,
                    func=mybir.ActivationFunctionType.Gelu,
                )
                nc.scalar.activation(
                    out=ht[:, 2 * GT : 4 * GT],
                    in_=ph_b,
                    func=mybir.ActivationFunctionType.Gelu,
                )

                # Keep the PE busy during gelu: L1 of the next iteration is
                # emitted (and thus executes) before L2 of this one.
                if idx + 1 < len(sched):
                    ph_cur = emit_l1(idx + 1)

                py = ps_y.tile([COUT, GT], F32, tag="py", name=f"py_{idx}")
                for oc in range(4):
                    nc.tensor.matmul(
                        py,
                        lhsT=w2_sb[t][:, oc, :],
                        rhs=ht[:, oc * GT : (oc + 1) * GT],
                        start=(oc == 0),
                        stop=(oc == 3),
                    )
                ysb = ypool.tile([COUT, GT], BF16, tag="ysb", name=f"ysb_{idx}")
                nc.vector.tensor_copy(out=ysb, in_=py)
                nc.sync.dma_start(out=yT[t][:, g * GT : (g + 1) * GT], in_=ysb)

    _split_matmul_waits(nc)
    return nc


def _split_matmul_waits(nc):
    """The fp32 self-loading matmul path has a single HW sync-wait slot and
    walrus cannot split multiple waits; hoist extras onto PE NoOps placed
    immediately before the matmul (same engine => program order preserved)."""
    for f in nc.m.functions:
        for bb in f.blocks:
            new = []
            changed = False
            for inst in bb.instructions:
                si = inst.sync_info
                if (
                    type(inst).__name__ != "InstNoOp"
                    and si is not None
                    and si.on_wait
                    and len(si.on_wait) > 1
                ):
                    waits = list(si.on_wait)
                    for w in waits[:-1]:
                        new.append(
                            mybir.InstNoOp(
                                name=nc.get_next_instruction_name(),
                                engine=inst.engine,
                                ins=[],
                                outs=[],
                                bass_nofuse=True,
                                sync_info=mybir.SyncInfo(on_wait=[w], on_update=[]),
                            )
                        )
                    inst.sync_info = mybir.SyncInfo(
                        on_wait=[waits[-1]], on_update=list(si.on_update)
                    )
                    changed = True
                new.append(inst)
            if changed:
                try:
                    bb.instructions[:] = new
                except TypeError:
                    bb.set_instructions(new)


def _prep_inputs(x, w1, w2):
    # xT[core][t, c, tok] = x[tok_global, c*16 + t]
    xr = np.asarray(x, dtype=np.float32).reshape(B_TOTAL, CIN, T).astype(NPBF16)
    xT = [
        np.ascontiguousarray(xr[c * B_CORE : (c + 1) * B_CORE].transpose(2, 1, 0))
        for c in range(NCORES)
    ]
    # W1_t^T [cin, cff] = w1[cff, cin, t]
    w1t = np.ascontiguousarray(w1.transpose(2, 1, 0).astype(NPBF16))
    # w2 tile [t, f', oc, cout] = w2[cout, 128*oc + f', t]
    w2t = np.ascontiguousarray(
        w2.transpose(2, 1, 0)
        .reshape(T, 4, 128, COUT)
        .transpose(0, 2, 1, 3)
        .astype(NPBF16)
    )
    return xT, w1t, w2t


_RESULT_CACHE = {}


def kernel(**inputs):
    x = np.asarray(inputs["x"])
    w1 = np.asarray(inputs["w1"])
    w2 = np.asarray(inputs["w2"])
    xT, w1t, w2t = _prep_inputs(x, w1, w2)

    nc = build_bass()
    in_maps = [
        {"xT": xT[c], "w1t": w1t, "w2t": w2t} for c in range(NCORES)
    ]
    res = bass_utils.run_bass_kernel_spmd(nc, in_maps, core_ids=list(range(NCORES)))
    if res.exec_time_ns is not None:
        print(f"HW exec time: {res.exec_time_ns} ns")
        _RESULT_CACHE["exec_time_ns"] = res.exec_time_ns
        _RESULT_CACHE["trace"] = res.instructions_and_trace
    # yT [t, o, tok] -> y[tok, o*16+t]
    y = np.concatenate(
        [
            np.asarray(res.results[c]["yT"])
            .transpose(2, 1, 0)
            .reshape(B_CORE, F)
            .astype(np.float32)
            for c in range(NCORES)
        ],
        axis=0,
    )
    return y.reshape(64, 256, F)


if __name__ == "__main__":
    rng = np.random.default_rng(0)
    x = rng.standard_normal((64, 256, 2048), dtype=np.float32)
    w1 = (rng.standard_normal((512, 128, 16), dtype=np.float32) * 0.05).astype(
        np.float32
    )
    w2 = (rng.standard_normal((128, 512, 16), dtype=np.float32) * 0.05).astype(
        np.float32
    )
    y = kernel(x=x, w1=w1, w2=w2)
    print("ok", y.shape, float(np.abs(y).mean()))


# revision 56
# speedup vs baseline: 2.0011x; 1.0846x over previous
"""JointWiseFeedForward Trainium2 kernel.

Computes, for each of T=16 token positions t (feature-interleaved, f = o*16+t):
    y[:, :, o*16+t] = gelu(x_t @ W1_t.T) @ W2_t.T        (exact erf gelu)
with x [64,256,2048] fp32, W1 [512,128,16], W2 [128,512,16].

Strategy: data-parallel over the flattened 16384-token axis across 8 cores
(2048 tokens/core); weights replicated.  The host pre-transposes x into
channel-major slabs xT[t, cin, tok] (bf16) so the device runs a pure
matmul/activation pipeline with zero PE transposes and zero layout shuffles:

The work per (t, 512-token group) iteration i is four "chunks" (oc 0..3):
    L1 chunk: one bf16 matmul (W1 chunk stationary) -> h^T slice in PSUM
    exact-GELU on ScalarE, PSUM -> SBUF bf16
    L2: 4 accumulating bf16 matmuls (W2 chunks stationary) -> y^T in PSUM
    DVE evicts y^T to SBUF bf16, DMA out channel-major; host re-interleaves.

The chunk stream is packed into [128, 1536] PSUM "generations" of 3 chunks
so each gelu is 1536 wide: the ScalarE's fixed ~222-cycle/instruction access
latency is amortized over 3 banks while PSUM still fits double buffering
(2x3 banks h + 2x1 bank y = 8 banks).  The dependency tracker hands pool
buffers off at whole-tile granularity, so the y tile must NOT share a PSUM
tile with h (the DVE eviction would serialize against the next-but-one L1).

Engine budget per core: ACT (gelu) 85*(1536+222)+(512+222) cyc @1.2GHz =
125us <- critical path; PE 262k cycles @2.4GHz = 109us; DVE 42us; DMA 21MB.
"""

import sys

import numpy as np

try:
    import concourse.bass as bass
except ImportError:  # fresh grading dir: repo lives at a fixed path in the image
    sys.path.insert(0, "/opt/trn_rl_repo")
    import concourse.bass as bass

import ml_dtypes
import concourse.mybir as mybir
import concourse.tile as tile
from concourse import bass_utils

NCORES = 8
B_TOTAL = 64 * 256          # 16384 flattened tokens
B_CORE = B_TOTAL // NCORES  # 2048
F = 2048
T = 16
CIN = 128                   # 2048 / 16
CFF = 512                   # 8192 / 16
COUT = 128
GT = 512                    # tokens per inner group (one PSUM bank of fp32)
GROUPS = B_CORE // GT       # 4
ITERS = T * GROUPS          # 64
NCHUNK = ITERS * 4          # 256 (iteration, oc) L1 chunks
# h generations are 3 PSUM banks each (2 buffers = 6 banks); the y
# accumulator is double-buffered (2 banks).  8 banks total.
GEN_SIZES = [3, 3]
PY_BUFS = 2

BF16 = mybir.dt.bfloat16
F32 = mybir.dt.float32
NPBF16 = ml_dtypes.bfloat16


def build_bass(n_iters: int = 1):
    nc = bass.Bass("TRN2")
    xT = nc.dram_tensor("xT", [T, CIN, B_CORE], BF16, kind="ExternalInput")
    w1t = nc.dram_tensor("w1t", [T, CIN, CFF], BF16, kind="ExternalInput")
    w2t = nc.dram_tensor("w2t", [T, 128, 4, COUT], BF16, kind="ExternalInput")
    yT = nc.dram_tensor("yT", [T, COUT, B_CORE], BF16, kind="ExternalOutput")

    with tile.TileContext(nc) as tc:
        with (
            tc.tile_pool(name="consts", bufs=1) as consts,
            tc.tile_pool(name="xg", bufs=2) as xpool,
            tc.tile_pool(
                name="htb", bufs=4
            ) as htpool,
            tc.tile_pool(
                name="ysb", bufs=4
            ) as ypool,
            tc.tile_pool(name="ps_h", bufs=1, space="PSUM") as ps_h,
            tc.tile_pool(name="ps_y", bufs=PY_BUFS, space="PSUM") as ps_y,
        ):
            w1_sb = {}
            w2_sb = {}
            xts = {}

            def load_w1(t):
                w1tile = consts.tile([CIN, CFF], BF16, tag=f"w1_{t}", name=f"w1s_{t}")
                nc.sync.dma_start(out=w1tile, in_=w1t[t])
                w1_sb[t] = w1tile

            def load_w2(t):
                w2tile = consts.tile([128, 4, COUT], BF16, tag=f"w2_{t}", name=f"w2s_{t}")
                nc.sync.dma_start(out=w2tile, in_=w2t[t])
                w2_sb[t] = w2tile

            def load_weights(t):
                load_w1(t)
                load_w2(t)

            def load_x(slab, split=1, quarters=None):
                # slab = global slab counter (t = slab % T); keyed globally so
                # multi-pass builds (n_iters > 1) get fresh pool tiles.
                if slab in xts:
                    xt_ = xts[slab]
                else:
                    xt_ = xpool.tile([CIN, B_CORE], BF16, tag="xg", name=f"xt_{slab}")
                    xts[slab] = xt_
                step = B_CORE // split
                q = nc.gpsimd  # SWDGE: runs parallel to the HWDGE weight/y DMAs
                for s in quarters if quarters is not None else range(split):
                    q.dma_start(
                        out=xt_[:, s * step : (s + 1) * step],
                        in_=xT[slab % T][:, s * step : (s + 1) * step],
                    )

            # Warm the Gelu activation table during the initial DMA wait so
            # the 1.3us table load is off the critical path.
            scratch = consts.tile([128, 1], F32, tag="scratch", name="scratch")
            dummy = consts.tile([128, 1], BF16, tag="dummy", name="dummy")
            nc.vector.memset(scratch, 0.0)
            nc.scalar.activation(
                out=dummy, in_=scratch, func=mybir.ActivationFunctionType.Gelu
            )

            # Preamble.  gen0 (chunk 0) needs only w1[0] and x0's first
            # quarter -- those two transfers go first, on separate DMA
            # generation devices (HWDGE / SWDGE) so they run in parallel.
            # gen2 starts iteration 1 (x0q1); w2[0] is not needed until the
            # first (lagged) L2.
            load_w1(0)
            load_x(0, split=4, quarters=[0])
            load_x(0, split=4, quarters=[1])
            load_w2(0)
            load_x(0, split=4, quarters=[2, 3])
            load_weights(1)
            load_weights(2)

            # Chunk c <-> (iteration i = c//4, oc = c%4); iteration i <->
            # (t = i//GROUPS, g = i%GROUPS).  Chunks are packed into 3-bank
            # PSUM generations; gelu runs once per generation.
            # ht_map[c] -> (ht tile, slot).
            nchunk = NCHUNK * n_iters
            ht_map = {}

            nslabs = T * n_iters

            def emit_l1_chunk(c, ph, slot):
                i, oc = (c // 4) % ITERS, c % 4
                t, g = i // GROUPS, i % GROUPS
                slab = c // (4 * GROUPS)
                if c % (4 * GROUPS) == 0:  # once per slab
                    if slab + 1 < nslabs:
                        load_x(slab + 1)
                    if c < NCHUNK and t + 3 < T:
                        load_weights(t + 3)
                nc.tensor.matmul(
                    ph[:, slot * GT : (slot + 1) * GT],
                    lhsT=w1_sb[t][:, oc * 128 : (oc + 1) * 128],
                    rhs=xts[slab][:, g * GT : (g + 1) * GT],
                    start=True,
                    stop=True,
                )

            def emit_l2(i):
                t, g = (i % ITERS) // GROUPS, (i % ITERS) % GROUPS
                py = ps_y.tile([COUT, GT], F32, tag="py", name=f"py_{i}")
                for oc in range(4):
                    ht, slot = ht_map.pop(4 * i + oc)
                    nc.tensor.matmul(
                        py,
                        lhsT=w2_sb[t][:, oc, :],
                        rhs=ht[:, slot * GT : (slot + 1) * GT],
                        start=(oc == 0),
                        stop=(oc == 3),
                    )
                ysb = ypool.tile([COUT, GT], BF16, tag="ysb", name=f"ysb_{i}")
                ydst = yT[t][:, g * GT : (g + 1) * GT]
                nc.vector.tensor_copy(out=ysb, in_=py)
                nc.sync.dma_start(out=ydst, in_=ysb)

            # Generation extents.  A [1, 2] prologue lets the first gelu
            # start after a single L1 chunk (one small DMA each of x and w1)
            # and ramps the ACT pipeline while the rest of the first x slab
            # is still in flight.
            sizes = [1, 2]
            while sum(sizes) + GEN_SIZES[len(sizes) % 2] <= nchunk:
                sizes.append(GEN_SIZES[len(sizes) % 2])
            if sum(sizes) < nchunk:
                sizes.append(nchunk - sum(sizes))
            gens = []
            lo = 0
            for w in sizes:
                gens.append((lo, lo + w))
                lo += w

            def emit_l1_gen(k):
                lo, hi = gens[k]
                ph = ps_h.tile(
                    [128, (hi - lo) * GT],
                    F32,
                    tag=f"ph{k % 2}",
                    name=f"ph_{k}",
                    bufs=1,
                )
                for c in range(lo, hi):
                    emit_l1_chunk(c, ph, c - lo)
                return ph

            # Software pipeline: gelu(k) | L1(k+1) | L2s covered by gen k-1.
            # The one-generation L2 lag keeps every PE instruction's waits
            # satisfied by the time the PE reaches it (an L2 emitted eagerly
            # would wait for gelu(k) mid-stream and push L1(k+1) past the
            # next gelu's start on short 3-bank periods).
            done_l2 = 0
            ph_cur = emit_l1_gen(0)
            for k in range(len(gens)):
                lo, hi = gens[k]
                ht = htpool.tile(
                    [128, (hi - lo) * GT], BF16, tag="ht", name=f"ht_{k}"
                )
                nc.scalar.activation(
                    out=ht, in_=ph_cur, func=mybir.ActivationFunctionType.Gelu
                )
                for c in range(lo, hi):
                    ht_map[c] = (ht, c - lo)
                if k + 1 < len(gens):
                    ph_cur = emit_l1_gen(k + 1)
                lag_hi = gens[k - 1][1] if k > 0 else 0
                while done_l2 * 4 + 3 < lag_hi:
                    emit_l2(done_l2)
                    done_l2 += 1
            while done_l2 * 4 < nchunk:
                emit_l2(done_l2)
                done_l2 += 1

    _split_matmul_waits(nc)
    return nc


def _split_matmul_waits(nc):
    """The fp32 self-loading matmul path has a single HW sync-wait slot and
    walrus cannot split multiple waits; hoist extras onto PE NoOps placed
    immediately before the matmul (same engine => program order preserved)."""
    for f in nc.m.functions:
        for bb in f.blocks:
            new = []
            changed = False
            for inst in bb.instructions:
                si = inst.sync_info
                if (
                    type(inst).__name__ != "InstNoOp"
                    and si is not None
                    and si.on_wait
                    and len(si.on_wait) > 1
                ):
                    waits = list(si.on_wait)
                    for w in waits[:-1]:
                        new.append(
                            mybir.InstNoOp(
                                name=nc.get_next_instruction_name(),
                                engine=inst.engine,
                                ins=[],
                                outs=[],
                                bass_nofuse=True,
                                sync_info=mybir.SyncInfo(on_wait=[w], on_update=[]),
                            )
                        )
                    inst.sync_info = mybir.SyncInfo(
                        on_wait=[waits[-1]], on_update=list(si.on_update)
                    )
                    changed = True
                new.append(inst)
            if changed:
                try:
                    bb.instructions[:] = new
                except TypeError:
                    bb.set_instructions(new)


def _prep_inputs(x, w1, w2):
    # xT[core][t, c, tok] = x[tok_global, c*16 + t]
    xr = np.asarray(x, dtype=np.float32).reshape(B_TOTAL, CIN, T).astype(NPBF16)
    xT = [
        np.ascontiguousarray(xr[c * B_CORE : (c + 1) * B_CORE].transpose(2, 1, 0))
        for c in range(NCORES)
    ]
    # W1_t^T [cin, cff] = w1[cff, cin, t]
    w1t = np.ascontiguousarray(w1.transpose(2, 1, 0).astype(NPBF16))
    # w2 tile [t, f', oc, cout] = w2[cout, 128*oc + f', t]
    w2t = np.ascontiguousarray(
        w2.transpose(2, 1, 0)
        .reshape(T, 4, 128, COUT)
        .transpose(0, 2, 1, 3)
        .astype(NPBF16)
    )
    return xT, w1t, w2t


_RESULT_CACHE = {}


def kernel(**inputs):
    x = np.asarray(inputs["x"])
    w1 = np.asarray(inputs["w1"])
    w2 = np.asarray(inputs["w2"])
    xT, w1t, w2t = _prep_inputs(x, w1, w2)

    nc = build_bass()
    in_maps = [
        {"xT": xT[c], "w1t": w1t, "w2t": w2t} for c in range(NCORES)
    ]
    res = bass_utils.run_bass_kernel_spmd(nc, in_maps, core_ids=list(range(NCORES)))
    if res.exec_time_ns is not None:
        print(f"HW exec time: {res.exec_time_ns} ns")
        _RESULT_CACHE["exec_time_ns"] = res.exec_time_ns
        _RESULT_CACHE["trace"] = res.instructions_and_trace
    # yT [t, o, tok] -> y[tok, o*16+t]
    y = np.concatenate(
        [
            np.asarray(res.results[c]["yT"])
            .transpose(2, 1, 0)
            .reshape(B_CORE, F)
            .astype(np.float32)
            for c in range(NCORES)
        ],
        axis=0,
    )
    return y.reshape(64, 256, F)


if __name__ == "__main__":
    rng = np.random.default_rng(0)
    x = rng.standard_normal((64, 256, 2048), dtype=np.float32)
    w1 = (rng.standard_normal((512, 128, 16), dtype=np.float32) * 0.05).astype(
        np.float32
    )
    w2 = (rng.standard_normal((128, 512, 16), dtype=np.float32) * 0.05).astype(
        np.float32
    )
    y = kernel(x=x, w1=w1, w2=w2)
    print("ok", y.shape, float(np.abs(y).mean()))
